# revision 14
# baseline (speedup 1.0000x reference)
"""AdaptiveClusteringAttention TRN2 kernel.

Data-parallel over batch: b=8 rows -> 8 NeuronCores, one row per core,
weights replicated. No collectives.

Per-core math (n=4096 tokens, d=1024, C=256 clusters, H=16 heads, dh=64):
  xc[c,:]   = sum_{t: cluster[t]=c} x[t,:]          (onehot matmul)
  cnt[c]    = |{t: cluster[t]=c}|
  xm[c,:]   = xc[c,:] / max(cnt[c], .5)
  kc        = xm @ w_k ; vc = xm @ w_v              (segmean commutes with proj)
  qh        = x @ w_q
  s[t,c]    = qh_h[t] . kc_h[c] / 8
  attn      = softmax(s + log cnt)                  (count-weighted softmax)
  out       = attn @ vc ; y = out @ w_proj + b_proj

Structure (v3):
- Phase A streams x per 512-token chunk (HWDGE f32 DMA + DVE bf16 cast),
  computes onehots/counts/cluster-sum partials (PSUM -> SBUF f32
  accumulate), stores x-bf16 to DRAM, reads it back transposed (XBAR),
  and runs the qh projection for the chunk so x DMA hides under qh
  matmuls.  wk/wv/wp cast-DMAs are gated on late-phase-A data (corner
  copies) so they don't congest the x-streaming DMA window.
- Phase B: kc^T and vc from the cluster means.
- Phase C per chunk: score matmuls K=64 write hh-pairs into a 2-bank
  PSUM tile so one exp activation (scale + log-count bias) evicts both;
  attn@vc with a ones column gives sum-exp for free; 1/sumexp rows are
  partition-broadcast on GpSimd (no PE); the previous chunk's output
  projection is interleaved into the current chunk's score loop to keep
  PE dense while ACT runs the exps.
"""

import os
import sys

import numpy as np

for _p in ("/opt/trn_rl_repo", os.path.expanduser("~/.axon_site/_ro/trn_rl_repo")):
    if os.path.isdir(_p) and _p not in sys.path:
        sys.path.append(_p)

import concourse.bass as bass  # noqa: E402
import concourse.mybir as mybir  # noqa: E402
import concourse.tile as tile  # noqa: E402
from concourse import bacc  # noqa: E402
from concourse.masks import make_identity  # noqa: E402

FP32 = mybir.dt.float32
BF16 = mybir.dt.bfloat16
I32 = mybir.dt.int32

N, D, C, H, DH, P = 4096, 1024, 256, 16, 64, 128
NJ = N // P          # 32 token row-tiles
NK = D // P          # 8 contraction chunks
TCH = 512            # token chunk
NCH = N // TCH       # 8 chunks
NMT = TCH // P       # 4 token subtiles per chunk
JPC = TCH // P       # 4 x row-tiles per chunk

TRACE = False
LAST_RESULTS = None


def build_nc():
    nc = bacc.Bacc("TRN2", target_bir_lowering=False, debug=False)

    x_d = nc.dram_tensor("x", [N, D], FP32, kind="ExternalInput").ap()
    cl_d = nc.dram_tensor("cluster", [N], I32, kind="ExternalInput").ap()
    wq_d = nc.dram_tensor("w_q", [D, D], FP32, kind="ExternalInput").ap()
    wk_d = nc.dram_tensor("w_k", [D, D], FP32, kind="ExternalInput").ap()
    wv_d = nc.dram_tensor("w_v", [D, D], FP32, kind="ExternalInput").ap()
    wp_d = nc.dram_tensor("w_proj", [D, D], FP32, kind="ExternalInput").ap()
    bp_d = nc.dram_tensor("b_proj", [1, D], FP32, kind="ExternalInput").ap()
    out_d = nc.dram_tensor("out", [N, D], FP32, kind="ExternalOutput").ap()

    with tile.TileContext(nc) as tc:
        with (
            tc.tile_pool(name="dram", bufs=1, space="DRAM") as dram,
            tc.tile_pool(name="wts", bufs=1) as wts,
        ):
            xbf_d = dram.tile([N, D], BF16)

            # ---- constants ----
            iota_b = wts.tile([P, C], BF16, tag="iota_b")
            ident = wts.tile([32, 32], BF16, tag="ident")
            make_identity(nc, ident[:])
            ones_col = wts.tile([P, 1], BF16, tag="ones_col")
            nc.vector.memset(ones_col[:], 1.0)
            ones_row = wts.tile([1, 64], BF16, tag="ones_row")
            nc.vector.memset(ones_row[:], 1.0)
            b_bc = wts.tile([P, D], FP32, tag="b_bc")
            clusT = wts.tile([P, NJ], FP32, tag="clusT")
            with (
                tc.tile_pool(name="boot", bufs=1) as boot,
                tc.tile_pool(name="psct", bufs=1, space="PSUM") as psct,
            ):
                iota_i = boot.tile([P, C], I32, tag="iota_i")
                nc.gpsimd.iota(iota_i[:], pattern=[[1, C]], base=0,
                               channel_multiplier=0)
                nc.vector.tensor_copy(iota_b[:], iota_i[:])
                bp_sb = boot.tile([1, D], FP32, tag="bp_sb")
                nc.sync.dma_start(out=bp_sb[:], in_=bp_d)
                nc.gpsimd.partition_broadcast(b_bc[:], bp_sb[:])
                cl_i = boot.tile([NJ, P], I32, tag="cl_i")
                nc.sync.dma_start(out=cl_i[:],
                                  in_=cl_d.rearrange("(a b) -> a b", b=P))
                cl_b = boot.tile([NJ, P], BF16, tag="cl_b")
                nc.vector.tensor_copy(cl_b[:], cl_i[:])
                ct_ps = psct.tile([P, NJ], BF16, tag="ct")
                nc.tensor.transpose(ct_ps[:], cl_b[:], ident[:])
                nc.vector.tensor_copy(clusT[:], ct_ps[:])

            wk_sb = [wts.tile([P, D], BF16, tag=f"wk{k}", name=f"wk{k}")
                     for k in range(NK)]
            wv_sb = [wts.tile([P, D], BF16, tag=f"wv{k}", name=f"wv{k}")
                     for k in range(NK)]
            wp_sb = [wts.tile([P, D], BF16, tag=f"wp{k}", name=f"wp{k}")
                     for k in range(NK)]

            # qh (d-major, bf16) for all chunks — phase A product
            qhall = [[wts.tile([P, TCH], BF16, tag=f"qh{ch}_{m}",
                               name=f"qh{ch}_{m}") for m in range(NK)]
                     for ch in range(NCH)]

            xcm = [wts.tile([P, C], BF16, tag=f"xcm{m}", name=f"xcm{m}")
                   for m in range(NK)]
            xc_acc = [wts.tile([P, C], FP32, tag=f"xca{m}", name=f"xca{m}")
                      for m in range(NK)]
            cnt_sb = wts.tile([1, C], FP32, tag="cnt_sb")
            logc = wts.tile([P, 2], FP32, tag="logc")
            inv_bc = wts.tile([P, C], FP32, tag="inv_bc")

            # ---- phase A: stream x; onehot/counts/cluster-sums + qh ----
            with (
                tc.tile_pool(name="psA", bufs=1, space="PSUM") as psA,
                tc.tile_pool(name="psxc", bufs=3, space="PSUM") as psxc,
                tc.tile_pool(name="psq", bufs=3, space="PSUM") as psq,
                tc.tile_pool(name="wqp", bufs=1) as wqp,
                tc.tile_pool(name="xf32", bufs=4) as xf32,
                tc.tile_pool(name="xin", bufs=6) as xin,
                tc.tile_pool(name="ohp", bufs=8) as ohp,
                tc.tile_pool(name="xtp", bufs=2) as xtp,
            ):
                # w_q (bf16 cast-DMA) — only needed during phase A
                wq_sb = []
                for k in range(NK):
                    t = wqp.tile([P, D], BF16, tag=f"wq{k}", name=f"wq{k}")
                    nc.gpsimd.dma_start(out=t[:],
                                        in_=wq_d[k * P:(k + 1) * P, :])
                    wq_sb.append(t)
                pcnt = psA.tile([1, C], FP32, tag="cnt")
                for ch in range(NCH):
                    t0 = ch * TCH
                    # gate deferred weight loads on late-phase-A data so
                    # their DMA doesn't congest the x-streaming window
                    if ch == 5:
                        for k in range(NK):
                            nc.vector.tensor_copy(wk_sb[k][0:1, 0:1],
                                                  qhall[4][7][0:1, 0:1])
                            nc.vector.tensor_copy(wv_sb[k][0:1, 0:1],
                                                  qhall[4][7][0:1, 0:1])
                            nc.gpsimd.dma_start(
                                out=wk_sb[k][:], in_=wk_d[k * P:(k + 1) * P, :])
                            nc.gpsimd.dma_start(
                                out=wv_sb[k][:], in_=wv_d[k * P:(k + 1) * P, :])
                    if ch == 7:
                        for k in range(NK):
                            nc.vector.tensor_copy(wp_sb[k][0:1, 0:1],
                                                  qhall[6][7][0:1, 0:1])
                            nc.gpsimd.dma_start(
                                out=wp_sb[k][:], in_=wp_d[k * P:(k + 1) * P, :])
                    ohc, xjc = [], []
                    for jj in range(JPC):
                        j = ch * JPC + jj
                        xf = xf32.tile([P, D], FP32, tag="xf")
                        nc.sync.dma_start(out=xf[:],
                                          in_=x_d[j * P:(j + 1) * P, :])
                        xj = xin.tile([P, D], BF16, tag="xj")
                        nc.vector.tensor_copy(xj[:], xf[:])
                        nc.scalar.dma_start(
                            out=xbf_d[j * P:(j + 1) * P, :], in_=xj[:]
                        )
                        oh = ohp.tile([P, C], BF16, tag="oh")
                        nc.vector.tensor_scalar(
                            oh[:], iota_b[:], clusT[:, j:j + 1], None,
                            mybir.AluOpType.is_equal,
                        )
                        nc.tensor.matmul(pcnt[:], ones_col[:], oh[:],
                                         start=(j == 0), stop=(j == NJ - 1))
                        ohc.append(oh)
                        xjc.append(xj)
                    # per-chunk cluster-sum partials -> SBUF f32 accumulate
                    for m in range(NK):
                        pxc = psxc.tile([P, C], FP32, tag="pxc")
                        for jj in range(JPC):
                            nc.tensor.matmul(
                                pxc[:], xjc[jj][:, m * P:(m + 1) * P],
                                ohc[jj][:],
                                start=(jj == 0), stop=(jj == JPC - 1),
                            )
                        if ch == 0:
                            nc.vector.tensor_copy(xc_acc[m][:], pxc[:])
                        else:
                            nc.vector.tensor_add(xc_acc[m][:], xc_acc[m][:],
                                                 pxc[:])
                    # transposed chunk via DRAM round trip (XBAR transpose)
                    xT = []
                    for k in range(NK):
                        t = xtp.tile([P, TCH], BF16, tag=f"xt{k}", name=f"xt{k}")
                        nc.sync.dma_start_transpose(
                            out=t[:], in_=xbf_d[t0:t0 + TCH, k * P:(k + 1) * P]
                        )
                        xT.append(t)
                    # qh projection for this chunk
                    for m in range(NK):
                        pq = psq.tile([P, TCH], FP32, tag="pq")
                        for k in range(NK):
                            nc.tensor.matmul(
                                pq[:], wq_sb[k][:, m * P:(m + 1) * P], xT[k][:],
                                start=(k == 0), stop=(k == NK - 1),
                            )
                        if m % 2 == 0:
                            nc.vector.tensor_copy(qhall[ch][m][:], pq[:])
                        else:
                            nc.scalar.copy(qhall[ch][m][:], pq[:])

                # counts -> inv (row + bcast); log-counts (column layout)
                nc.scalar.copy(cnt_sb[:], pcnt[:])
                cm_row = wts.tile([1, C], FP32, tag="cm_row")
                nc.vector.tensor_scalar_max(cm_row[:], cnt_sb[:], 0.5)
                inv_row = wts.tile([1, C], FP32, tag="inv_row")
                nc.vector.reciprocal(inv_row[:], cm_row[:])
                nc.gpsimd.partition_broadcast(inv_bc[:], inv_row[:])

                cnt_col = wts.tile([P, 2], FP32, tag="cnt_col")
                for mc in range(2):
                    nc.gpsimd.dma_start(
                        out=cnt_col[:, mc:mc + 1],
                        in_=cnt_sb[0:1, mc * P:(mc + 1) * P],
                    )
                cm_col = wts.tile([P, 2], FP32, tag="cm_col")
                nc.vector.tensor_scalar_max(cm_col[:], cnt_col[:], 0.5)
                lg_col = wts.tile([P, 2], FP32, tag="lg_col")
                nc.scalar.activation(lg_col[:], cm_col[:],
                                     mybir.ActivationFunctionType.Ln)
                msk = wts.tile([P, 2], FP32, tag="msk")
                nc.vector.tensor_scalar(
                    msk[:], cnt_col[:], 0.5, 30.0,
                    mybir.AluOpType.is_lt, mybir.AluOpType.mult,
                )
                nc.vector.tensor_sub(logc[:], lg_col[:], msk[:])

                # xm^T = xc^T * inv  (d-major cluster means)
                for m in range(NK):
                    nc.vector.tensor_mul(xcm[m][:], xc_acc[m][:], inv_bc[:])

            # ---- phase B: kc^T and vc (with ones column) ----
            kc_sb = [wts.tile([P, C], BF16, tag=f"kc{m}", name=f"kc{m}")
                     for m in range(NK)]
            vca = [wts.tile([P, 16 * 65], BF16, tag=f"vca{i}", name=f"vca{i}")
                   for i in range(2)]
            for i in range(2):
                va = vca[i].rearrange("p (h e) -> p h e", e=65)
                nc.vector.memset(va[:, :, 64:65], 1.0)
            with (
                tc.tile_pool(name="psBk", bufs=2, space="PSUM") as psBk,
                tc.tile_pool(name="psBv", bufs=4, space="PSUM") as psBv,
            ):
                for m in range(NK):
                    pk = psBk.tile([P, C], FP32, tag="pk")
                    for k in range(NK):
                        nc.tensor.matmul(
                            pk[:], wk_sb[k][:, m * P:(m + 1) * P],
                            xcm[k][:], start=(k == 0), stop=(k == NK - 1),
                        )
                    nc.vector.tensor_copy(kc_sb[m][:], pk[:])
                for mc in range(2):
                    va = vca[mc].rearrange("p (h e) -> p h e", e=65)
                    for nn in range(2):
                        pv = psBv.tile([P, 512], FP32, tag="pv")
                        for k in range(NK):
                            nc.tensor.matmul(
                                pv[:], xcm[k][:, mc * P:(mc + 1) * P],
                                wv_sb[k][:, nn * 512:(nn + 1) * 512],
                                start=(k == 0), stop=(k == NK - 1),
                            )
                        nc.vector.tensor_copy(
                            va[:, nn * 8:(nn + 1) * 8, 0:64],
                            pv.rearrange("p (h e) -> p h e", e=64),
                        )

            # ---- phase C: per token-chunk attention + output proj ----
            # chunk ch's scores/exp/attn run interleaved with chunk ch-1's
            # output projection so PE stays dense while ACT runs exps
            with (
                tc.tile_pool(name="expp", bufs=3) as expp,
                tc.tile_pool(name="sep", bufs=1) as sep,
                tc.tile_pool(name="otp", bufs=2) as otp,
                tc.tile_pool(name="finp", bufs=2) as finp,
                tc.tile_pool(name="pss", bufs=2, space="PSUM") as pss,
                tc.tile_pool(name="psav", bufs=2, space="PSUM") as psav,
                tc.tile_pool(name="psf", bufs=2, space="PSUM") as psf,
            ):
                outT_prev = None
                outT = None
                se_eo = None
                pend_av = []

                def emit_av(mm, exs):
                    for hh in range(2):
                        h = 2 * mm + hh
                        off = hh * 64
                        pav = psav.tile([65, TCH], FP32, tag="av")
                        for mc in range(2):
                            nc.tensor.matmul(
                                pav[:], vca[mc][:, h * 65:(h + 1) * 65],
                                exs[mc][:, hh * TCH:(hh + 1) * TCH],
                                start=(mc == 0), stop=(mc == 1),
                            )
                        se_dst = se_eo[hh][0:1, mm * TCH:(mm + 1) * TCH]
                        if hh == 0:
                            nc.vector.tensor_copy(se_dst, pav[64:65, :])
                            nc.scalar.copy(outT[mm][off:off + 64, :],
                                           pav[0:64, :])
                        else:
                            nc.scalar.copy(se_dst, pav[64:65, :])
                            nc.vector.tensor_copy(
                                outT[mm][off:off + 64, :], pav[0:64, :])

                for ch in range(NCH + 1):
                    cur = ch if ch < NCH else None
                    if cur is not None:
                        qh = qhall[cur]
                        outT = [otp.tile([P, TCH], BF16, tag=f"ot{m}",
                                         name=f"ot{m}") for m in range(NK)]
                        se_eo = [sep.tile([1, (H // 2) * TCH], BF16,
                                          tag=f"se{i}", name=f"se{i}")
                                 for i in range(2)]
                        pend_av = []
                    for m in range(NK):
                        if cur is not None:
                            # scores for head pair (2m, 2m+1): hh halves go
                            # to the two banks of one PSUM tile; adjacent
                            # matmuls use different PE row groups
                            exs = []
                            pss_t = []
                            for mc in range(2):
                                ps2 = pss.tile([P, 2 * TCH], FP32, tag="s2")
                                for hh in range(2):
                                    off = hh * 64
                                    nc.tensor.matmul(
                                        ps2[:, hh * TCH:(hh + 1) * TCH],
                                        kc_sb[m][off:off + 64,
                                                 mc * P:(mc + 1) * P],
                                        qh[m][off:off + 64, :],
                                        start=True, stop=True,
                                    )
                                pss_t.append(ps2)
                            for mc in range(2):
                                e2t = expp.tile([P, 2 * TCH], BF16, tag="exp")
                                nc.scalar.activation(
                                    e2t[:], pss_t[mc][:],
                                    mybir.ActivationFunctionType.Exp,
                                    bias=logc[:, mc:mc + 1], scale=0.125,
                                )
                                exs.append(e2t)
                            pend_av.append((m, exs))
                            if len(pend_av) >= 2:
                                emit_av(*pend_av.pop(0))
                        if ch > 0:
                            # previous chunk's projection slice
                            mt, nn = m // 2, m % 2
                            t0p = (ch - 1) * TCH
                            pf = psf.tile([P, 512], FP32, tag="pf")
                            for k in range(NK):
                                nc.tensor.matmul(
                                    pf[:],
                                    outT_prev[k][:, mt * P:(mt + 1) * P],
                                    wp_sb[k][:, nn * 512:(nn + 1) * 512],
                                    start=(k == 0), stop=(k == NK - 1),
                                )
                            fin = finp.tile([P, 512], FP32, tag="fin")
                            nc.vector.tensor_add(
                                fin[:], pf[:], b_bc[:, nn * 512:(nn + 1) * 512]
                            )
                            nc.gpsimd.dma_start(
                                out=out_d[t0p + mt * P:t0p + (mt + 1) * P,
                                          nn * 512:(nn + 1) * 512],
                                in_=fin[:],
                            )
                    if cur is not None:
                        while pend_av:
                            emit_av(*pend_av.pop(0))
                        # 1/sumexp: gather, batched reciprocal, scatter to
                        # bf16 rows, then col-group-paired K=1 matmuls
                        # broadcast each head's row to 64 partitions
                        sq = sep.tile([P, TCH // 8], FP32, tag="sq")
                        for i in range(2):
                            nc.gpsimd.dma_start(
                                out=sq[i * 64:(i + 1) * 64, :],
                                in_=se_eo[i].rearrange("a (p t) -> a p t",
                                                       t=TCH),
                            )
                        rq = sep.tile([P, TCH // 8], FP32, tag="rq")
                        nc.vector.reciprocal(rq[:], sq[:])
                        rec_eo = [sep.tile([1, (H // 2) * TCH], BF16,
                                           tag=f"rec{i}", name=f"rec{i}")
                                  for i in range(2)]
                        for i in range(2):
                            nc.gpsimd.dma_start(
                                out=rec_eo[i].rearrange("a (p t) -> a p t",
                                                        t=TCH),
                                in_=rq[i * 64:(i + 1) * 64, :],
                            )
                        for m in range(NK):
                            pbc = psf.tile([P, TCH], FP32, tag="pf")
                            for i in range(2):
                                nc.tensor.matmul(
                                    pbc[i * 64:(i + 1) * 64, :], ones_row[:],
                                    rec_eo[i][0:1, m * TCH:(m + 1) * TCH],
                                    start=True, stop=True,
                                )
                            nc.vector.tensor_mul(outT[m][:], outT[m][:],
                                                 pbc[:])
                        outT_prev = outT
    nc.compile()
    return nc


_NC = None


def _get_nc():
    global _NC
    if _NC is None:
        _NC = build_nc()
    return _NC


def make_in_maps(cluster, q, w_q, w_kv, w_proj, b_proj):
    cluster = np.ascontiguousarray(np.asarray(cluster).astype(np.int32, copy=False))
    q = np.asarray(q, dtype=np.float32)
    w_q = np.ascontiguousarray(np.asarray(w_q, dtype=np.float32))
    w_kv = np.asarray(w_kv, dtype=np.float32)
    w_k = np.ascontiguousarray(w_kv[:, :D])
    w_v = np.ascontiguousarray(w_kv[:, D:])
    w_proj = np.ascontiguousarray(np.asarray(w_proj, dtype=np.float32))
    b_proj = np.ascontiguousarray(
        np.asarray(b_proj, dtype=np.float32).reshape(1, D)
    )
    return [
        {
            "x": np.ascontiguousarray(q[i]),
            "cluster": cluster[i],
            "w_q": w_q,
            "w_k": w_k,
            "w_v": w_v,
            "w_proj": w_proj,
            "b_proj": b_proj,
        }
        for i in range(q.shape[0])
    ]


def kernel(cluster, q, w_q, w_kv, w_proj, b_proj):
    global LAST_RESULTS
    from concourse.bass_utils import run_bass_kernel_spmd

    nc = _get_nc()
    in_maps = make_in_maps(cluster, q, w_q, w_kv, w_proj, b_proj)
    ncores = len(in_maps)
    res = run_bass_kernel_spmd(
        nc, in_maps, core_ids=list(range(ncores)), trace=TRACE
    )
    LAST_RESULTS = res
    return np.stack([res.results[i]["out"] for i in range(ncores)], axis=0)


# revision 17
# speedup vs baseline: 1.1364x; 1.1364x over previous
"""AdaptiveClusteringAttention TRN2 kernel.

Data-parallel over batch: b=8 rows -> 8 NeuronCores, one row per core,
weights replicated. No collectives.

Per-core math (n=4096 tokens, d=1024, C=256 clusters, H=16 heads, dh=64):
  xc[c,:]   = sum_{t: cluster[t]=c} x[t,:]          (onehot matmul)
  cnt[c]    = |{t: cluster[t]=c}|
  xm[c,:]   = xc[c,:] / max(cnt[c], .5)
  kc        = xm @ w_k ; vc = xm @ w_v              (segmean commutes with proj)
  qh        = x @ w_q
  s[t,c]    = qh_h[t] . kc_h[c] / 8
  attn      = softmax(s + log cnt)                  (count-weighted softmax)
  out       = attn @ vc ; y = out @ w_proj + b_proj

Structure (v3):
- Phase A streams x per 512-token chunk (HWDGE f32 DMA + DVE bf16 cast),
  computes onehots/counts/cluster-sum partials (PSUM -> SBUF f32
  accumulate), stores x-bf16 to DRAM, reads it back transposed (XBAR),
  and runs the qh projection for the chunk so x DMA hides under qh
  matmuls.  wk/wv/wp cast-DMAs are gated on late-phase-A data (corner
  copies) so they don't congest the x-streaming DMA window.
- Phase B: kc^T and vc from the cluster means.
- Phase C per chunk: score matmuls K=64 write hh-pairs into a 2-bank
  PSUM tile so one exp activation (scale + log-count bias) evicts both;
  attn@vc with a ones column gives sum-exp for free; 1/sumexp rows are
  partition-broadcast on GpSimd (no PE); the previous chunk's output
  projection is interleaved into the current chunk's score loop to keep
  PE dense while ACT runs the exps.
"""

import os
import sys

import numpy as np

for _p in ("/opt/trn_rl_repo", os.path.expanduser("~/.axon_site/_ro/trn_rl_repo")):
    if os.path.isdir(_p) and _p not in sys.path:
        sys.path.append(_p)

import concourse.bass as bass  # noqa: E402
import concourse.mybir as mybir  # noqa: E402
import concourse.tile as tile  # noqa: E402
from concourse import bacc  # noqa: E402
from concourse.masks import make_identity  # noqa: E402

FP32 = mybir.dt.float32
BF16 = mybir.dt.bfloat16
I32 = mybir.dt.int32

N, D, C, H, DH, P = 4096, 1024, 256, 16, 64, 128
NJ = N // P          # 32 token row-tiles
NK = D // P          # 8 contraction chunks
TCH = 512            # token chunk
NCH = N // TCH       # 8 chunks
NMT = TCH // P       # 4 token subtiles per chunk
JPC = TCH // P       # 4 x row-tiles per chunk

TRACE = False
LAST_RESULTS = None


def build_nc():
    nc = bacc.Bacc("TRN2", target_bir_lowering=False, debug=False)

    x_d = nc.dram_tensor("x", [N, D], FP32, kind="ExternalInput").ap()
    cl_d = nc.dram_tensor("cluster", [N], I32, kind="ExternalInput").ap()
    wq_d = nc.dram_tensor("w_q", [D, D], FP32, kind="ExternalInput").ap()
    wk_d = nc.dram_tensor("w_k", [D, D], FP32, kind="ExternalInput").ap()
    wv_d = nc.dram_tensor("w_v", [D, D], FP32, kind="ExternalInput").ap()
    wp_d = nc.dram_tensor("w_proj", [D, D], FP32, kind="ExternalInput").ap()
    bp_d = nc.dram_tensor("b_proj", [1, D], FP32, kind="ExternalInput").ap()
    out_d = nc.dram_tensor("out", [N, D], FP32, kind="ExternalOutput").ap()

    with tile.TileContext(nc) as tc:
        with (
            tc.tile_pool(name="dram", bufs=1, space="DRAM") as dram,
            tc.tile_pool(name="wts", bufs=1) as wts,
        ):
            xbf_d = dram.tile([N, D], BF16)
            rec_d = [dram.tile([2, (H // 2) * TCH], BF16, tag=f"rec{i}",
                               name=f"rec{i}") for i in range(2)]

            # ---- constants ----
            iota_b = wts.tile([P, C], BF16, tag="iota_b")
            ident = wts.tile([32, 32], BF16, tag="ident")
            make_identity(nc, ident[:])
            ones_col = wts.tile([P, 1], BF16, tag="ones_col")
            nc.vector.memset(ones_col[:], 1.0)
            b_bc = wts.tile([P, D], FP32, tag="b_bc")
            clusT = wts.tile([P, NJ], FP32, tag="clusT")
            with (
                tc.tile_pool(name="boot", bufs=1) as boot,
                tc.tile_pool(name="psct", bufs=1, space="PSUM") as psct,
            ):
                iota_i = boot.tile([P, C], I32, tag="iota_i")
                nc.gpsimd.iota(iota_i[:], pattern=[[1, C]], base=0,
                               channel_multiplier=0)
                nc.vector.tensor_copy(iota_b[:], iota_i[:])
                bp_sb = boot.tile([1, D], FP32, tag="bp_sb")
                nc.sync.dma_start(out=bp_sb[:], in_=bp_d)
                nc.gpsimd.partition_broadcast(b_bc[:], bp_sb[:])
                cl_i = boot.tile([NJ, P], I32, tag="cl_i")
                nc.sync.dma_start(out=cl_i[:],
                                  in_=cl_d.rearrange("(a b) -> a b", b=P))
                cl_b = boot.tile([NJ, P], BF16, tag="cl_b")
                nc.vector.tensor_copy(cl_b[:], cl_i[:])
                ct_ps = psct.tile([P, NJ], BF16, tag="ct")
                nc.tensor.transpose(ct_ps[:], cl_b[:], ident[:])
                nc.vector.tensor_copy(clusT[:], ct_ps[:])

            wk_sb = [wts.tile([P, D], BF16, tag=f"wk{k}", name=f"wk{k}")
                     for k in range(NK)]
            wv_sb = [wts.tile([P, D], BF16, tag=f"wv{k}", name=f"wv{k}")
                     for k in range(NK)]
            wp_sb = [wts.tile([P, D], BF16, tag=f"wp{k}", name=f"wp{k}")
                     for k in range(NK)]

            # qh (d-major, bf16) for all chunks — phase A product
            qhall = [[wts.tile([P, TCH], BF16, tag=f"qh{ch}_{m}",
                               name=f"qh{ch}_{m}") for m in range(NK)]
                     for ch in range(NCH)]

            xcm = [wts.tile([P, C], BF16, tag=f"xcm{m}", name=f"xcm{m}")
                   for m in range(NK)]
            xc_acc = [wts.tile([P, C], FP32, tag=f"xca{m}", name=f"xca{m}")
                      for m in range(NK)]
            cnt_sb = wts.tile([1, C], FP32, tag="cnt_sb")
            logc = wts.tile([P, 2], FP32, tag="logc")
            inv_bc = wts.tile([P, C], FP32, tag="inv_bc")

            # ---- phase A: stream x; onehot/counts/cluster-sums + qh ----
            with (
                tc.tile_pool(name="psA", bufs=1, space="PSUM") as psA,
                tc.tile_pool(name="psxc", bufs=3, space="PSUM") as psxc,
                tc.tile_pool(name="psq", bufs=3, space="PSUM") as psq,
                tc.tile_pool(name="wqp", bufs=1) as wqp,
                tc.tile_pool(name="xin", bufs=10) as xin,
                tc.tile_pool(name="ohp", bufs=8) as ohp,
                tc.tile_pool(name="xtp", bufs=2) as xtp,
            ):
                # w_q (bf16 cast-DMA) — only needed during phase A
                wq_sb = []
                for k in range(NK):
                    t = wqp.tile([P, D], BF16, tag=f"wq{k}", name=f"wq{k}")
                    nc.gpsimd.dma_start(out=t[:],
                                        in_=wq_d[k * P:(k + 1) * P, :])
                    wq_sb.append(t)
                pcnt = psA.tile([1, C], FP32, tag="cnt")
                for ch in range(NCH):
                    t0 = ch * TCH
                    # gate deferred weight loads on late-phase-A data so
                    # their DMA doesn't congest the x-streaming window
                    if ch == 5:
                        for k in range(NK):
                            nc.vector.tensor_copy(wk_sb[k][0:1, 0:1],
                                                  qhall[4][7][0:1, 0:1])
                            nc.vector.tensor_copy(wv_sb[k][0:1, 0:1],
                                                  qhall[4][7][0:1, 0:1])
                            nc.gpsimd.dma_start(
                                out=wk_sb[k][:], in_=wk_d[k * P:(k + 1) * P, :])
                            nc.gpsimd.dma_start(
                                out=wv_sb[k][:], in_=wv_d[k * P:(k + 1) * P, :])
                    if ch == 7:
                        for k in range(NK):
                            nc.vector.tensor_copy(wp_sb[k][0:1, 0:1],
                                                  qhall[6][7][0:1, 0:1])
                            nc.gpsimd.dma_start(
                                out=wp_sb[k][:], in_=wp_d[k * P:(k + 1) * P, :])
                    ohc, xjc = [], []
                    for jj in range(JPC):
                        j = ch * JPC + jj
                        xj = xin.tile([P, D], BF16, tag="xj")
                        nc.gpsimd.dma_start(out=xj[:],
                                            in_=x_d[j * P:(j + 1) * P, :])
                        nc.scalar.dma_start(
                            out=xbf_d[j * P:(j + 1) * P, :], in_=xj[:])
                        oh = ohp.tile([P, C], BF16, tag="oh")
                        nc.vector.tensor_scalar(
                            oh[:], iota_b[:], clusT[:, j:j + 1], None,
                            mybir.AluOpType.is_equal,
                        )
                        nc.tensor.matmul(pcnt[:], ones_col[:], oh[:],
                                         start=(j == 0), stop=(j == NJ - 1))
                        ohc.append(oh)
                        xjc.append(xj)
                    # per-chunk cluster-sum partials -> SBUF f32 accumulate
                    for m in range(NK):
                        pxc = psxc.tile([P, C], FP32, tag="pxc")
                        for jj in range(JPC):
                            nc.tensor.matmul(
                                pxc[:], xjc[jj][:, m * P:(m + 1) * P],
                                ohc[jj][:],
                                start=(jj == 0), stop=(jj == JPC - 1),
                            )
                        if ch == 0:
                            nc.vector.tensor_copy(xc_acc[m][:], pxc[:])
                        else:
                            nc.vector.tensor_add(xc_acc[m][:], xc_acc[m][:],
                                                 pxc[:])
                    # transposed chunk via DRAM round trip; sync engine only
                    xT = []
                    for k in range(NK):
                        t = xtp.tile([P, TCH], BF16, tag=f"xt{k}", name=f"xt{k}")
                        nc.sync.dma_start_transpose(
                            out=t[:], in_=xbf_d[t0:t0 + TCH, k * P:(k + 1) * P]
                        )
                        xT.append(t)
                    # qh projection for this chunk
                    for m in range(NK):
                        pq = psq.tile([P, TCH], FP32, tag="pq")
                        for k in range(NK):
                            nc.tensor.matmul(
                                pq[:], wq_sb[k][:, m * P:(m + 1) * P], xT[k][:],
                                start=(k == 0), stop=(k == NK - 1),
                            )
                        if m % 2 == 0:
                            nc.vector.tensor_copy(qhall[ch][m][:], pq[:])
                        else:
                            nc.scalar.copy(qhall[ch][m][:], pq[:])

                # counts -> inv (row + bcast); log-counts (column layout)
                nc.scalar.copy(cnt_sb[:], pcnt[:])
                cm_row = wts.tile([1, C], FP32, tag="cm_row")
                nc.vector.tensor_scalar_max(cm_row[:], cnt_sb[:], 0.5)
                inv_row = wts.tile([1, C], FP32, tag="inv_row")
                nc.vector.reciprocal(inv_row[:], cm_row[:])
                nc.gpsimd.partition_broadcast(inv_bc[:], inv_row[:])

                cnt_col = wts.tile([P, 2], FP32, tag="cnt_col")
                for mc in range(2):
                    nc.gpsimd.dma_start(
                        out=cnt_col[:, mc:mc + 1],
                        in_=cnt_sb[0:1, mc * P:(mc + 1) * P],
                    )
                cm_col = wts.tile([P, 2], FP32, tag="cm_col")
                nc.vector.tensor_scalar_max(cm_col[:], cnt_col[:], 0.5)
                lg_col = wts.tile([P, 2], FP32, tag="lg_col")
                nc.scalar.activation(lg_col[:], cm_col[:],
                                     mybir.ActivationFunctionType.Ln)
                msk = wts.tile([P, 2], FP32, tag="msk")
                nc.vector.tensor_scalar(
                    msk[:], cnt_col[:], 0.5, 30.0,
                    mybir.AluOpType.is_lt, mybir.AluOpType.mult,
                )
                nc.vector.tensor_sub(logc[:], lg_col[:], msk[:])

                # xm^T = xc^T * inv  (d-major cluster means)
                for m in range(NK):
                    nc.vector.tensor_mul(xcm[m][:], xc_acc[m][:], inv_bc[:])

            # ---- phase B: kc^T and vc (with ones column) ----
            kc_sb = [wts.tile([P, C], BF16, tag=f"kc{m}", name=f"kc{m}")
                     for m in range(NK)]
            vca = [wts.tile([P, 16 * 65], BF16, tag=f"vca{i}", name=f"vca{i}")
                   for i in range(2)]
            for i in range(2):
                va = vca[i].rearrange("p (h e) -> p h e", e=65)
                nc.vector.memset(va[:, :, 64:65], 1.0)
            with (
                tc.tile_pool(name="psBk", bufs=2, space="PSUM") as psBk,
                tc.tile_pool(name="psBv", bufs=4, space="PSUM") as psBv,
            ):
                for m in range(NK):
                    pk = psBk.tile([P, C], FP32, tag="pk")
                    for k in range(NK):
                        nc.tensor.matmul(
                            pk[:], wk_sb[k][:, m * P:(m + 1) * P],
                            xcm[k][:], start=(k == 0), stop=(k == NK - 1),
                        )
                    nc.vector.tensor_copy(kc_sb[m][:], pk[:])
                for mc in range(2):
                    va = vca[mc].rearrange("p (h e) -> p h e", e=65)
                    for nn in range(2):
                        pv = psBv.tile([P, 512], FP32, tag="pv")
                        for k in range(NK):
                            nc.tensor.matmul(
                                pv[:], xcm[k][:, mc * P:(mc + 1) * P],
                                wv_sb[k][:, nn * 512:(nn + 1) * 512],
                                start=(k == 0), stop=(k == NK - 1),
                            )
                        nc.vector.tensor_copy(
                            va[:, nn * 8:(nn + 1) * 8, 0:64],
                            pv.rearrange("p (h e) -> p h e", e=64),
                        )

            # ---- phase C: per token-chunk attention + output proj ----
            # chunk ch's scores/exp/attn run interleaved with chunk ch-1's
            # output projection so PE stays dense while ACT runs exps
            with (
                tc.tile_pool(name="expp", bufs=4) as expp,
                tc.tile_pool(name="sep", bufs=1) as sep,
                tc.tile_pool(name="bcp", bufs=2) as bcp,
                tc.tile_pool(name="otp", bufs=2) as otp,
                tc.tile_pool(name="finp", bufs=2) as finp,
                tc.tile_pool(name="pss", bufs=2, space="PSUM") as pss,
                tc.tile_pool(name="psav", bufs=2, space="PSUM") as psav,
                tc.tile_pool(name="psf", bufs=2, space="PSUM") as psf,
            ):
                outT_prev = None
                outT = None
                se_eo = None
                pend_av = []

                def emit_av(mm, exs):
                    for hh in range(2):
                        h = 2 * mm + hh
                        off = hh * 64
                        pav = psav.tile([65, TCH], FP32, tag="av")
                        for mc in range(2):
                            nc.tensor.matmul(
                                pav[:], vca[mc][:, h * 65:(h + 1) * 65],
                                exs[mc][:, hh * TCH:(hh + 1) * TCH],
                                start=(mc == 0), stop=(mc == 1),
                            )
                        se_dst = se_eo[hh][0:1, mm * TCH:(mm + 1) * TCH]
                        nc.vector.tensor_copy(se_dst, pav[64:65, :])
                        if hh == 0:
                            nc.scalar.copy(outT[mm][off:off + 64, :],
                                           pav[0:64, :])
                        else:
                            nc.vector.tensor_copy(
                                outT[mm][off:off + 64, :], pav[0:64, :])

                for ch in range(NCH + 1):
                    cur = ch if ch < NCH else None
                    if cur is not None:
                        qh = qhall[cur]
                        outT = [otp.tile([P, TCH], BF16, tag=f"ot{m}",
                                         name=f"ot{m}") for m in range(NK)]
                        se_eo = [sep.tile([1, (H // 2) * TCH], BF16,
                                          tag=f"se{i}", name=f"se{i}")
                                 for i in range(2)]
                        pend_av = []
                    for m in range(NK):
                        if cur is not None:
                            # scores for head pair (2m, 2m+1): hh halves go
                            # to the two banks of one PSUM tile; adjacent
                            # matmuls use different PE row groups
                            exs = []
                            pss_t = []
                            for mc in range(2):
                                ps2 = pss.tile([P, 2 * TCH], FP32, tag="s2")
                                for hh in range(2):
                                    off = hh * 64
                                    nc.tensor.matmul(
                                        ps2[:, hh * TCH:(hh + 1) * TCH],
                                        kc_sb[m][off:off + 64,
                                                 mc * P:(mc + 1) * P],
                                        qh[m][off:off + 64, :],
                                        start=True, stop=True,
                                    )
                                pss_t.append(ps2)
                            for mc in range(2):
                                e2t = expp.tile([P, 2 * TCH], BF16, tag="exp")
                                nc.scalar.activation(
                                    e2t[:], pss_t[mc][:],
                                    mybir.ActivationFunctionType.Exp,
                                    bias=logc[:, mc:mc + 1], scale=0.125,
                                )
                                exs.append(e2t)
                            pend_av.append((m, exs))
                            if len(pend_av) >= 2:
                                emit_av(*pend_av.pop(0))
                        if ch > 0:
                            # previous chunk's projection slice
                            mt, nn = m // 2, m % 2
                            t0p = (ch - 1) * TCH
                            pf = psf.tile([P, 512], FP32, tag="pf")
                            for k in range(NK):
                                nc.tensor.matmul(
                                    pf[:],
                                    outT_prev[k][:, mt * P:(mt + 1) * P],
                                    wp_sb[k][:, nn * 512:(nn + 1) * 512],
                                    start=(k == 0), stop=(k == NK - 1),
                                )
                            fin = finp.tile([P, 512], FP32, tag="fin")
                            nc.vector.tensor_add(
                                fin[:], pf[:], b_bc[:, nn * 512:(nn + 1) * 512]
                            )
                            nc.gpsimd.dma_start(
                                out=out_d[t0p + mt * P:t0p + (mt + 1) * P,
                                          nn * 512:(nn + 1) * 512],
                                in_=fin[:],
                            )
                    if cur is not None:
                        while pend_av:
                            emit_av(*pend_av.pop(0))
                        # 1/sumexp: gather, batched reciprocal, scatter to
                        # bf16 rows, then col-group-paired K=1 matmuls
                        # broadcast each head's row to 64 partitions
                        sq = sep.tile([P, TCH // 8], FP32, tag="sq")
                        for i in range(2):
                            nc.gpsimd.dma_start(
                                out=sq[i * 64:(i + 1) * 64, :],
                                in_=se_eo[i].rearrange("a (p t) -> a p t",
                                                       t=TCH),
                            )
                        rq = sep.tile([P, TCH // 8], FP32, tag="rq")
                        nc.vector.reciprocal(rq[:], sq[:])
                        rd = rec_d[ch % 2]
                        for i in range(2):
                            nc.gpsimd.dma_start(
                                out=rd[i:i + 1, :].rearrange(
                                    "a (p t) -> a p t", t=TCH),
                                in_=rq[i * 64:(i + 1) * 64, :],
                            )
                        bcall = bcp.tile([P, (H // 2) * TCH], BF16, tag="bc")
                        for i in range(2):
                            nc.sync.dma_start(
                                out=bcall[i * 64:(i + 1) * 64, :],
                                in_=rd[i:i + 1, :].partition_broadcast(64),
                            )
                        for m in range(NK):
                            nc.vector.tensor_mul(
                                outT[m][:], outT[m][:],
                                bcall[:, m * TCH:(m + 1) * TCH])
                        outT_prev = outT
    nc.compile()
    return nc


_NC = None


def _get_nc():
    global _NC
    if _NC is None:
        _NC = build_nc()
    return _NC


def make_in_maps(cluster, q, w_q, w_kv, w_proj, b_proj):
    cluster = np.ascontiguousarray(np.asarray(cluster).astype(np.int32, copy=False))
    q = np.asarray(q, dtype=np.float32)
    w_q = np.ascontiguousarray(np.asarray(w_q, dtype=np.float32))
    w_kv = np.asarray(w_kv, dtype=np.float32)
    w_k = np.ascontiguousarray(w_kv[:, :D])
    w_v = np.ascontiguousarray(w_kv[:, D:])
    w_proj = np.ascontiguousarray(np.asarray(w_proj, dtype=np.float32))
    b_proj = np.ascontiguousarray(
        np.asarray(b_proj, dtype=np.float32).reshape(1, D)
    )
    return [
        {
            "x": np.ascontiguousarray(q[i]),
            "cluster": cluster[i],
            "w_q": w_q,
            "w_k": w_k,
            "w_v": w_v,
            "w_proj": w_proj,
            "b_proj": b_proj,
        }
        for i in range(q.shape[0])
    ]


def kernel(cluster, q, w_q, w_kv, w_proj, b_proj):
    global LAST_RESULTS
    from concourse.bass_utils import run_bass_kernel_spmd

    nc = _get_nc()
    in_maps = make_in_maps(cluster, q, w_q, w_kv, w_proj, b_proj)
    ncores = len(in_maps)
    res = run_bass_kernel_spmd(
        nc, in_maps, core_ids=list(range(ncores)), trace=TRACE
    )
    LAST_RESULTS = res
    return np.stack([res.results[i]["out"] for i in range(ncores)], axis=0)
